# revision 1
# baseline (speedup 1.0000x reference)
"""Bass/Trainium2 kernel for nn_F_Loss_65446711656630.

Strategy (data-parallel over N, 8 cores):
  - Host: GLOBAL stable sort of all rows by class id, then slice 8192 rows
    per core and transpose to [512 features x 8192 rows] contiguous pieces.
    After a global sort each core spans only ~2 classes, so class
    transitions are rare at any granularity.
  - Device (static kernel): stream 16 fp16 pieces of [128, 2048]; per piece
      * DVE:  per-128-row-chunk partial sums of h (one multi-chunk
              TensorReduce per piece, 3D access pattern)
      * ACT:  square with accum_out -> per-piece partial sum of h^2
              (the square pass itself yields the sumsq reduction, so no
              second DVE reduce pass is needed)
    Inputs ship as fp16 (costs ~1e-5 final rel err, halves HBM traffic to
    8 MiB/core); the per-element engine passes (~36-39 us) are the cap,
    with DMA at ~23 us well underneath.
  - Host: per-class stats from single-class chunk/piece partials (fp64)
    + direct numpy sums for the few transition chunks/pieces; then the
    tiny O(C^2 D) pairwise betainc/top-k stage in f32 jax on CPU
    (mirroring the reference's numerics exactly).
"""

import numpy as np

C = 16
D = 512
N = 65536
NCORES = 8
ROWS = N // NCORES          # 8192 rows per core
P = 128                     # SBUF partitions
PIECE = 2048                # rows per DMA piece / sumsq granule
X = 128                     # rows per sums granule (DVE reduce chunk)
NBLK = D // P               # 4 feature blocks
NPIECE = ROWS // PIECE      # 4 pieces per block
NCHUNK = ROWS // X          # 64 chunks per core
CPP = PIECE // X            # 8 chunks per piece
XMIN, XMAX = 1e-37, 1.0 - 1e-5

_NC_CACHE = {}


def _build_nc():
    """Per-core SPMD program.

    Inputs:  "ht"   [16, 128, 2048] fp16 (piece (b,p) at index b*4+p holds
                                         features b*128..+128 x rows
                                         p*2048..+2048, contiguous)
    Outputs: "hsum" [128, 256] f32  (hsum[f, b*64+g] = sum over 128-row
                                     chunk g of feature b*128+f)
             "ssum" [128, 16]  f32  (ssum[f, b*4+p]  = sum over piece p's
                                     2048 rows of feature (b*128+f)^2)
    """
    import concourse.tile as tile
    from concourse import bacc, mybir

    f32 = mybir.dt.float32

    nc = bacc.Bacc("TRN2", target_bir_lowering=False, debug=False,
                   num_devices=NCORES)
    f16 = mybir.dt.float16
    ht = nc.declare_dram_parameter("ht", [NBLK * NPIECE, P, PIECE], f16,
                                   isOutput=False)
    hsum = nc.declare_dram_parameter("hsum", [P, NBLK * NCHUNK], f32, isOutput=True)
    ssum = nc.declare_dram_parameter("ssum", [P, NBLK * NPIECE], f32, isOutput=True)

    with tile.TileContext(nc) as tc:
        with (
            tc.tile_pool(name="pc", bufs=8) as piece_pool,
            tc.tile_pool(name="sq", bufs=3) as sq_pool,
            tc.tile_pool(name="acc", bufs=1) as acc_pool,
        ):
            hpart = acc_pool.tile([P, NBLK * NCHUNK], f32, tag="hpart")
            spart = acc_pool.tile([P, NBLK * NPIECE], f32, tag="spart")

            for i in range(NBLK * NPIECE):
                t = piece_pool.tile([P, PIECE], f16)
                nc.sync.dma_start(t[:], ht[i])

                # ACT: square (scratch) + free-dim accumulate -> piece sumsq
                sq = sq_pool.tile([P, PIECE], f32)
                nc.scalar.activation(
                    sq[:], t[:], mybir.ActivationFunctionType.Square,
                    accum_out=spart[:, i:i + 1])

                # DVE: one multi-chunk reduce -> chunk sums of h
                base = i * CPP
                t3 = t[:].rearrange("p (c x) -> p c x", x=X)
                nc.vector.reduce_sum(
                    hpart[:, base:base + CPP], t3, axis=mybir.AxisListType.X)

            nc.sync.dma_start(hsum[:], hpart[:])
            nc.sync.dma_start(ssum[:], spart[:])
    nc.compile()
    return nc


def _get_nc():
    if "nc" not in _NC_CACHE:
        _NC_CACHE["nc"] = _build_nc()
    return _NC_CACHE["nc"]


def _granule_classes(ids_sorted, size):
    """Per-granule class id, or -1 if the granule spans a class boundary."""
    g = ids_sorted.reshape(-1, size)
    pure = g[:, 0] == g[:, -1]
    return np.where(pure, g[:, 0], -1).astype(np.int64)


def _prep_core(hs_k, ids_k):
    """hs_k/ids_k already globally sorted. Returns device input + host fixups."""
    T = np.ascontiguousarray(
        hs_k.reshape(NPIECE, PIECE, NBLK, P).transpose(2, 0, 3, 1)
        .astype(np.float16)
    ).reshape(NBLK * NPIECE, P, PIECE)           # [16, 128, 2048] fp16

    chunk_cls = _granule_classes(ids_k, X)       # [64]
    piece_cls = _granule_classes(ids_k, PIECE)   # [8]

    bsum = np.zeros((C, D), dtype=np.float64)
    bsq = np.zeros((C, D), dtype=np.float64)
    # transition chunks: host computes their per-class h sums
    if (chunk_cls < 0).any():
        m = np.repeat(chunk_cls < 0, X)
        rows, rids = hs_k[m].astype(np.float64), ids_k[m]
        for q in np.unique(rids):
            bsum[q] += rows[rids == q].sum(axis=0)
    # transition pieces: host computes their per-class h^2 sums
    if (piece_cls < 0).any():
        m = np.repeat(piece_cls < 0, PIECE)
        rows, rids = hs_k[m].astype(np.float64), ids_k[m]
        for q in np.unique(rids):
            sel = rows[rids == q]
            bsq[q] += (sel * sel).sum(axis=0)
    return T, chunk_cls, piece_cls, bsum, bsq


def _device_stats(hidden, ids, **run_kwargs):
    """Returns (sums[C,D], sumsq[C,D]) float64, plus the raw run result."""
    from concourse import bass_utils

    nc = _get_nc()

    order = np.argsort(ids, kind="stable")       # GLOBAL sort by class
    ids_s = ids[order]
    hs = hidden[order]

    in_maps = []
    meta = []
    sums = np.zeros((C, D), dtype=np.float64)
    sumsq = np.zeros((C, D), dtype=np.float64)
    for k in range(NCORES):
        rows = slice(k * ROWS, (k + 1) * ROWS)
        T, ccls, pcls, bsum, bsq = _prep_core(hs[rows], ids_s[rows])
        in_maps.append({"ht": T})
        meta.append((ccls, pcls))
        sums += bsum
        sumsq += bsq

    res = bass_utils.run_bass_kernel_spmd(nc, in_maps, list(range(NCORES)), **run_kwargs)

    eye = np.arange(C)[None, :]
    for k in range(NCORES):
        ccls, pcls = meta[k]
        hp = res.results[k]["hsum"].astype(np.float64)
        sp = res.results[k]["ssum"].astype(np.float64)
        # [128, b, g] -> [g, b, 128] -> [granule, feature]
        hp = hp.reshape(P, NBLK, NCHUNK).transpose(2, 1, 0).reshape(NCHUNK, D)
        sp = sp.reshape(P, NBLK, NPIECE).transpose(2, 1, 0).reshape(NPIECE, D)
        cm = ccls >= 0
        sums += ((ccls[cm, None] == eye).astype(np.float64)).T @ hp[cm]
        pm = pcls >= 0
        sumsq += ((pcls[pm, None] == eye).astype(np.float64)).T @ sp[pm]
    return sums, sumsq, res


def _pairwise_loss(counts, sums, sumsq, d):
    """The tiny O(C^2 D) stage on host CPU.

    Runs in float32 with the same jax ops as the reference: at these extreme
    betainc parameters (b ~ 8190, x ~ 1e-5) jax's f32 betainc differs from
    the true (f64) value by ~1e-3, so matching the reference requires
    replicating its f32 numerics, not improving on them.
    """
    import jax
    import jax.numpy as jnp

    cpu = jax.devices("cpu")[0]
    with jax.default_device(cpu):
        counts64 = counts.astype(np.float64)
        means64 = sums / counts64[:, None]
        withins64 = sumsq - counts64[:, None] * means64**2
        counts = jnp.asarray(counts64, jnp.float32)               # [C]
        means = jnp.asarray(means64, jnp.float32)                 # [C, D]
        withins = jnp.asarray(withins64, jnp.float32)             # [C, D]
        half_diff = (means[:, None, :] - means[None, :, :]) * 0.5
        pair_counts = counts[:, None] + counts[None, :]
        pair_between = half_diff * half_diff * pair_counts[:, :, None]
        pair_within = withins[:, None, :] + withins[None, :, :]
        d2 = pair_counts - 2.0
        d2 = jnp.where(d2 == 0.0, 1e-5, d2)
        x = pair_between / (pair_between + pair_within)
        x = jnp.clip(x, XMIN, XMAX)
        a = jnp.full_like(x, 0.5)
        b = jnp.broadcast_to((d2 * 0.5)[:, :, None], x.shape)
        xbetainc = jax.scipy.special.betainc(a, b, x)             # [C, C, D]
        top_k, _ = jax.lax.top_k(xbetainc, int(d))                # [C, C, d]
        per_pair = jnp.sum(jnp.log(top_k), axis=-1)               # [C, C]
        mask = jnp.triu(jnp.ones((C, C), dtype=bool), k=1)
        total = jnp.sum(jnp.where(mask, per_pair, jnp.zeros_like(per_pair)))
        return float(-total)


def kernel(hidden, batch_ids, d):
    hidden = np.asarray(hidden, dtype=np.float32)
    ids = np.asarray(batch_ids).astype(np.int64)
    assert hidden.shape == (N, D), hidden.shape

    counts = np.bincount(ids, minlength=C).astype(np.float64)
    sums, sumsq, _ = _device_stats(hidden, ids)
    total = _pairwise_loss(counts, sums, sumsq, int(np.asarray(d)))
    return np.array(total, dtype=np.float32)



# revision 3
# speedup vs baseline: 1.2413x; 1.2413x over previous
"""Bass/Trainium2 kernel for nn_F_Loss_65446711656630.

Strategy (data-parallel over N, 8 cores):
  - Host: GLOBAL stable sort of all rows by class id, then slice 8192 rows
    per core and transpose to [512 features x 8192 rows] contiguous pieces
    (layout: partitions = features, free axis = rows).
  - Device (static kernel): stream 16 pieces of [128, 2048]; pieces are
    statically split between the two element-wise engines so both finish
    together:
      * DVE pieces ('V'): 4x bn_stats per piece (512-row groups). One DVE
        pass yields count/mean/M2 for even+odd lanes -> BOTH the h-sum and
        the h^2-sum for that granule. This is the key trick: the vector
        engine computes both statistics in a single pass over the data.
      * ACT pieces ('A'): Copy activation with accum_out (-> per-piece h
        sums) + Square activation with accum_out (-> per-piece h^2 sums).
      * One piece ('S') is split between the engines to fine-balance.
    Inputs ship as fp8 e4m3 when DT_IN='fp8' (final rel err ~2e-4, well
    inside the 2e-2 gate) halving HBM traffic to 4 MiB/core; fp16 fallback.
  - Host: per-class stats from pure-class granules (f64 accumulation)
    + direct numpy sums for the few granules spanning a class boundary;
    then the tiny O(C^2 D) pairwise betainc/top-k stage in f32 jax on CPU
    (mirroring the reference's numerics exactly).
"""

import numpy as np

C = 16
D = 512
N = 65536
NCORES = 8
ROWS = N // NCORES          # 8192 rows per core
P = 128                     # SBUF partitions
PIECE = 2048                # rows per DMA piece
G = 512                     # bn_stats group size (HW max)
GPP = PIECE // G            # 4 groups per piece
NBLK = D // P               # 4 feature blocks
NPIECE = ROWS // PIECE      # 4 row-pieces per core
NP_TOT = NBLK * NPIECE      # 16 pieces per core
XMIN, XMAX = 1e-37, 1.0 - 1e-5

# Engine assignment per piece: V=DVE bn_stats, A=ACT copy/square+accum,
# S=split (DVE groups 0-1, ACT rows 1024:2048).
ASSIGN = "VVAVVAVVAVAVVAVS"
assert len(ASSIGN) == NP_TOT

DT_IN = "fp8"               # "fp8" (e4m3) or "fp16"

_NC_CACHE = {}


def _np_in_dtype():
    if DT_IN == "fp8":
        import ml_dtypes
        return ml_dtypes.float8_e4m3fn
    return np.float16


def _build_nc():
    """Per-core SPMD program.

    Inputs:  "ht"    [16, 128, 2048] (piece i = b*4+p holds features
                                      b*128..+128 x rows p*2048..+2048)
    Outputs: "bn"    [128, 384] f32  (bn[f, i*24+g*6 : +6] = bn_stats of
                                      piece i group g: [ce,me,M2e,co,mo,M2o])
             "hacc"  [128, 16]  f32  (ACT pieces: per-piece h sums)
             "sqacc" [128, 16]  f32  (ACT pieces: per-piece h^2 sums)
    """
    import concourse.tile as tile
    from concourse import bacc, mybir

    f32 = mybir.dt.float32
    dt_in = mybir.dt.float8e4 if DT_IN == "fp8" else mybir.dt.float16
    AF = mybir.ActivationFunctionType

    nc = bacc.Bacc("TRN2", target_bir_lowering=False, debug=False,
                   num_devices=NCORES)
    ht = nc.declare_dram_parameter("ht", [NP_TOT, P, PIECE], dt_in,
                                   isOutput=False)
    bn = nc.declare_dram_parameter("bn", [P, NP_TOT * GPP * 6], f32,
                                   isOutput=True)
    hacc = nc.declare_dram_parameter("hacc", [P, NP_TOT], f32, isOutput=True)
    sqacc = nc.declare_dram_parameter("sqacc", [P, NP_TOT], f32, isOutput=True)

    with tile.TileContext(nc) as tc:
        with (
            tc.tile_pool(name="pc", bufs=NP_TOT) as piece_pool,
            tc.tile_pool(name="acc", bufs=1) as acc_pool,
        ):
            bnT = acc_pool.tile([P, NP_TOT * GPP * 6], f32, tag="bn")
            haccT = acc_pool.tile([P, NP_TOT], f32, tag="hacc")
            sqaccT = acc_pool.tile([P, NP_TOT], f32, tag="sqacc")
            cp_scr = acc_pool.tile([P, PIECE], mybir.dt.float16, tag="cps")
            sq_scr = acc_pool.tile([P, PIECE], mybir.dt.float16, tag="sqs")

            # zero the accumulators (only assigned slices get written);
            # runs before the first DMA lands, so it's free
            nc.vector.memset(bnT[:], 0.0)
            nc.scalar.memzero(haccT[:])
            nc.scalar.memzero(sqaccT[:])

            for i, a in enumerate(ASSIGN):
                t = piece_pool.tile([P, PIECE], dt_in)
                nc.sync.dma_start(t[:], ht[i])

                if a == "V":
                    t3 = t[:].rearrange("p (g x) -> p g x", x=G)
                    for g in range(GPP):
                        nc.vector.bn_stats(
                            bnT[:, (i * GPP + g) * 6:(i * GPP + g + 1) * 6],
                            t3[:, g, :])
                elif a == "A":
                    nc.scalar.activation(
                        cp_scr[:], t[:], AF.Copy,
                        accum_out=haccT[:, i:i + 1])
                    nc.scalar.activation(
                        sq_scr[:], t[:], AF.Square,
                        accum_out=sqaccT[:, i:i + 1])
                else:  # "S": DVE takes groups 0-1, ACT takes rows 1024:
                    t3 = t[:].rearrange("p (g x) -> p g x", x=G)
                    for g in range(2):
                        nc.vector.bn_stats(
                            bnT[:, (i * GPP + g) * 6:(i * GPP + g + 1) * 6],
                            t3[:, g, :])
                    half = slice(PIECE // 2, PIECE)
                    nc.scalar.activation(
                        cp_scr[:, 0:PIECE // 2], t[:, half], AF.Copy,
                        accum_out=haccT[:, i:i + 1])
                    nc.scalar.activation(
                        sq_scr[:, 0:PIECE // 2], t[:, half], AF.Square,
                        accum_out=sqaccT[:, i:i + 1])

            nc.sync.dma_start(bn[:], bnT[:])
            nc.sync.dma_start(hacc[:], haccT[:])
            nc.sync.dma_start(sqacc[:], sqaccT[:])
    nc.compile()
    return nc


def _get_nc():
    if "nc" not in _NC_CACHE:
        _NC_CACHE["nc"] = _build_nc()
    return _NC_CACHE["nc"]


def _piece_granules(i):
    """(row_offset_within_piece, length, source) for piece i."""
    a = ASSIGN[i]
    if a == "V":
        return [(g * G, G, "bn") for g in range(GPP)]
    if a == "A":
        return [(0, PIECE, "acc")]
    return [(0, G, "bn"), (G, G, "bn"), (PIECE // 2, PIECE // 2, "acc")]


def _prep_core(hs_k):
    """hs_k already globally sorted. Returns device input [16, 128, 2048]."""
    return np.ascontiguousarray(
        hs_k.reshape(NPIECE, PIECE, NBLK, P).transpose(2, 0, 3, 1)
        .astype(_np_in_dtype())
    ).reshape(NP_TOT, P, PIECE)


def _core_stats(hs_k, ids_k, dev, sums, sumsq):
    """Accumulate per-class stats for one core into sums/sumsq [C, D] f64.

    Pure-class granules use device stats; granules spanning a class
    boundary are recomputed exactly on the host from the raw f32 rows.
    """
    bnr = dev["bn"].astype(np.float64).reshape(P, NP_TOT, GPP, 6)
    ha = dev["hacc"].astype(np.float64)
    sq = dev["sqacc"].astype(np.float64)
    for i in range(NP_TOT):
        b, p = divmod(i, NPIECE)
        fsl = slice(b * P, (b + 1) * P)
        for off, ln, src in _piece_granules(i):
            r0 = p * PIECE + off
            r1 = r0 + ln
            if ids_k[r0] == ids_k[r1 - 1]:
                c = int(ids_k[r0])
                if src == "bn":
                    ce, me, m2e, co, mo, m2o = bnr[:, i, off // G, :].T
                    sums[c, fsl] += ce * me + co * mo
                    sumsq[c, fsl] += m2e + ce * me * me + m2o + co * mo * mo
                else:
                    sums[c, fsl] += ha[:, i]
                    sumsq[c, fsl] += sq[:, i]
            else:
                rows = hs_k[r0:r1, fsl].astype(np.float64)
                rids = ids_k[r0:r1]
                for q in np.unique(rids):
                    sel = rows[rids == q]
                    sums[q, fsl] += sel.sum(axis=0)
                    sumsq[q, fsl] += (sel * sel).sum(axis=0)


def _device_stats(hidden, ids, **run_kwargs):
    """Returns (sums[C,D], sumsq[C,D]) float64, plus the raw run result."""
    from concourse import bass_utils

    nc = _get_nc()

    order = np.argsort(ids, kind="stable")       # GLOBAL sort by class
    ids_s = ids[order]
    hs = hidden[order]

    in_maps = []
    for k in range(NCORES):
        rows = slice(k * ROWS, (k + 1) * ROWS)
        in_maps.append({"ht": _prep_core(hs[rows])})

    res = bass_utils.run_bass_kernel_spmd(nc, in_maps, list(range(NCORES)),
                                          **run_kwargs)

    sums = np.zeros((C, D), dtype=np.float64)
    sumsq = np.zeros((C, D), dtype=np.float64)
    for k in range(NCORES):
        rows = slice(k * ROWS, (k + 1) * ROWS)
        _core_stats(hs[rows], ids_s[rows], res.results[k], sums, sumsq)
    return sums, sumsq, res


def _pairwise_loss(counts, sums, sumsq, d):
    """The tiny O(C^2 D) stage on host CPU.

    Runs in float32 with the same jax ops as the reference: at these extreme
    betainc parameters (b ~ 8190, x ~ 1e-5) jax's f32 betainc differs from
    the true (f64) value by ~1e-3, so matching the reference requires
    replicating its f32 numerics, not improving on them.
    """
    import jax
    import jax.numpy as jnp

    cpu = jax.devices("cpu")[0]
    with jax.default_device(cpu):
        counts64 = counts.astype(np.float64)
        means64 = sums / counts64[:, None]
        withins64 = sumsq - counts64[:, None] * means64**2
        counts = jnp.asarray(counts64, jnp.float32)               # [C]
        means = jnp.asarray(means64, jnp.float32)                 # [C, D]
        withins = jnp.asarray(withins64, jnp.float32)             # [C, D]
        half_diff = (means[:, None, :] - means[None, :, :]) * 0.5
        pair_counts = counts[:, None] + counts[None, :]
        pair_between = half_diff * half_diff * pair_counts[:, :, None]
        pair_within = withins[:, None, :] + withins[None, :, :]
        d2 = pair_counts - 2.0
        d2 = jnp.where(d2 == 0.0, 1e-5, d2)
        x = pair_between / (pair_between + pair_within)
        x = jnp.clip(x, XMIN, XMAX)
        a = jnp.full_like(x, 0.5)
        b = jnp.broadcast_to((d2 * 0.5)[:, :, None], x.shape)
        xbetainc = jax.scipy.special.betainc(a, b, x)             # [C, C, D]
        top_k, _ = jax.lax.top_k(xbetainc, int(d))                # [C, C, d]
        per_pair = jnp.sum(jnp.log(top_k), axis=-1)               # [C, C]
        mask = jnp.triu(jnp.ones((C, C), dtype=bool), k=1)
        total = jnp.sum(jnp.where(mask, per_pair, jnp.zeros_like(per_pair)))
        return float(-total)


def kernel(hidden, batch_ids, d):
    hidden = np.asarray(hidden, dtype=np.float32)
    ids = np.asarray(batch_ids).astype(np.int64)
    assert hidden.shape == (N, D), hidden.shape

    counts = np.bincount(ids, minlength=C).astype(np.float64)
    sums, sumsq, _ = _device_stats(hidden, ids)
    total = _pairwise_loss(counts, sums, sumsq, int(np.asarray(d)))
    return np.array(total, dtype=np.float32)
